# revision 12
# baseline (speedup 1.0000x reference)
"""Haar wavelet (2x2 stride-2, per-channel) Trainium2 Bass kernel.

Full input x: (8, 64, 512, 512) f32 -> full output (8, 256, 256, 256) f32.
Sharding: pure data parallel over batch -- core i processes x[i].

Per-core kernel layout (C=64 channels, H=W=512):
  - Block = KC channels x 128 output rows. One load DMA per block
    (128, KC*1024): partition p holds input rows (2*(i0+p), 2*(i0+p)+1)
    -- 4KB contiguous runs in DRAM -- for KC channels.
  - Halve in place (DVE tensor_scalar 2x), then
  - Vertical butterfly (DVE): s = top + bot ; d = bot - top
  - Horizontal butterfly (DVE, stride-2 reads):
      ll = s_e + s_o ; lh = d_e + d_o ; hl = s_o - s_e ; hh = d_o - d_e
  - One store DMA per block: the block's 4*KC output channels are
    contiguous in DRAM (channel layout [c*(ll,lh,hl,hh)]).
Engine roles: ACT = load ring, SP = store ring, DVE = all compute.
All compute on one engine keeps every instruction at <=2 sync waits
(the walrus codegen limit). Emission is software-pipelined (stage2 of
block i-1 after stage1 of block i) so no engine idles.
"""

import sys

if "/opt/trn_rl_repo" not in sys.path:
    sys.path.insert(0, "/opt/trn_rl_repo")

from contextlib import ExitStack

import numpy as np

import concourse.bass as bass
import concourse.tile as tile
from concourse import bacc
from concourse import mybir
from concourse.bass_utils import run_bass_kernel_spmd

N_CORES = 8
C, H, W = 64, 512, 512
F32 = mybir.dt.float32
ADD = mybir.AluOpType.add
SUB = mybir.AluOpType.subtract

_CACHED = {}


def _build(C=C, H=H, W=W, KC=4, P=128):
    HO, WO = H // 2, W // 2
    N_HB = HO // P
    nc = bacc.Bacc("TRN2", target_bir_lowering=False, debug=False)
    x = nc.dram_tensor("x", [C, H, W], F32, kind="ExternalInput").ap()
    out = nc.dram_tensor("out", [4 * C, HO, WO], F32, kind="ExternalOutput").ap()

    blocks = [(cg * KC, hb * P) for cg in range(C // KC) for hb in range(N_HB)]

    with tile.TileContext(nc) as tc, ExitStack() as ctx:
        xpool = ctx.enter_context(tc.tile_pool(name="xp", bufs=5))
        mpool = ctx.enter_context(tc.tile_pool(name="mid", bufs=2))
        rpool = ctx.enter_context(tc.tile_pool(name="raw", bufs=4))

        pending = None  # (s_t, d_t, c0, i0) awaiting stage2 + halve + store

        def stage2_and_store(s_t, d_t, c0, i0):
            s2 = s_t[:].rearrange("p (k j t) -> p k j t", k=KC, t=2)
            d2 = d_t[:].rearrange("p (k j t) -> p k j t", k=KC, t=2)
            s_e, s_o = s2[:, :, :, 0], s2[:, :, :, 1]
            d_e, d_o = d2[:, :, :, 0], d2[:, :, :, 1]

            rt = rpool.tile([P, KC * 4 * WO], F32)
            r4 = rt[:].rearrange("p (k q j) -> p k q j", k=KC, q=4)
            nc.vector.tensor_tensor(r4[:, :, 0, :], s_e, s_o, ADD)  # ll
            nc.vector.tensor_tensor(r4[:, :, 1, :], d_e, d_o, ADD)  # lh
            nc.vector.tensor_tensor(r4[:, :, 2, :], s_o, s_e, SUB)  # hl
            nc.vector.tensor_tensor(r4[:, :, 3, :], d_o, d_e, SUB)  # hh

            # One store DMA: the block's 4*KC output channels are
            # contiguous in DRAM.
            dst = out[4 * c0 : 4 * (c0 + KC), i0 : i0 + P, :].transpose([1, 0, 2])
            nc.sync.dma_start(dst, rt[:].rearrange("p (c j) -> p c j", j=WO))

        def emit_load(c0, i0):
            # load: (128, KC, 1024); p holds rows 2*(i0+p), 2*(i0+p)+1
            xt = xpool.tile([P, KC * 2 * W], F32, name="xt")
            src = x[c0 : c0 + KC, 2 * i0 : 2 * i0 + 2 * P, :].rearrange(
                "k (p t) w -> p k (t w)", t=2
            )
            nc.scalar.dma_start(xt[:].rearrange("p (k f) -> p k f", k=KC), src)
            return xt

        # issue loads two blocks ahead so the ACT halve (which waits on
        # the load's completion) never starves the load ring
        xts = [emit_load(*blocks[0]), emit_load(*blocks[1])]
        for idx, (c0, i0) in enumerate(blocks):
            xt = xts.pop(0)
            if idx + 2 < len(blocks):
                xts.append(emit_load(*blocks[idx + 2]))

            # ---- halve in place on ACT (frees ~70us of DVE time)
            nc.scalar.mul(xt[:], xt[:], 0.5)

            x4 = xt[:].rearrange("p (k t w) -> p k t w", k=KC, t=2)
            top, bot = x4[:, :, 0, :], x4[:, :, 1, :]

            # ---- vertical butterfly (DVE)
            s_t = mpool.tile([P, KC * W], F32)
            d_t = mpool.tile([P, KC * W], F32)
            sv = s_t[:].rearrange("p (k w) -> p k w", k=KC)
            dv = d_t[:].rearrange("p (k w) -> p k w", k=KC)
            nc.vector.tensor_tensor(sv, top, bot, ADD)
            nc.vector.tensor_tensor(dv, bot, top, SUB)

            # ---- previous block's stage2 + halve + store (software pipelining)
            if pending is not None:
                stage2_and_store(*pending)
            pending = (s_t, d_t, c0, i0)

        stage2_and_store(*pending)
    nc.compile()
    return nc


def _get_nc():
    if "nc" not in _CACHED:
        _CACHED["nc"] = _build()
    return _CACHED["nc"]


def _run(x, **kwargs):
    x = np.ascontiguousarray(np.asarray(x), dtype=np.float32)
    assert x.shape == (N_CORES, C, H, W), x.shape
    nc = _get_nc()
    in_maps = [{"x": np.ascontiguousarray(x[i])} for i in range(N_CORES)]
    res = run_bass_kernel_spmd(nc, in_maps, core_ids=list(range(N_CORES)), **kwargs)
    out = np.stack([res.results[i]["out"] for i in range(N_CORES)], axis=0)
    return out, res


def kernel(x):
    return _run(x)[0]


# revision 16
# speedup vs baseline: 1.1371x; 1.1371x over previous
"""Haar wavelet (2x2 stride-2, per-channel) Trainium2 Bass kernel.

Full input x: (8, 64, 512, 512) f32 -> full output (8, 256, 256, 256) f32.
Sharding: pure data parallel over batch -- core i processes x[i].

Per-core kernel layout (C=64 channels, H=W=512):
  - Block = KC channels x 128 output rows. One load DMA per block
    (128, KC*1024): partition p holds input rows (2*(i0+p), 2*(i0+p)+1)
    -- 4KB contiguous runs in DRAM -- for KC channels.
  - Halve in place (DVE tensor_scalar 2x), then
  - Vertical butterfly (DVE): s = top + bot ; d = bot - top
  - Horizontal butterfly (DVE, stride-2 reads):
      ll = s_e + s_o ; lh = d_e + d_o ; hl = s_o - s_e ; hh = d_o - d_e
  - One store DMA per block: the block's 4*KC output channels are
    contiguous in DRAM (channel layout [c*(ll,lh,hl,hh)]).
Engine roles: ACT = load ring, SP = store ring, DVE = all compute.
All compute on one engine keeps every instruction at <=2 sync waits
(the walrus codegen limit). Emission is software-pipelined (stage2 of
block i-1 after stage1 of block i) so no engine idles.
"""

import sys

if "/opt/trn_rl_repo" not in sys.path:
    sys.path.insert(0, "/opt/trn_rl_repo")

from contextlib import ExitStack

import numpy as np

import concourse.bass as bass
import concourse.tile as tile
from concourse import bacc
from concourse import mybir
from concourse.bass_utils import run_bass_kernel_spmd

N_CORES = 8
C, H, W = 64, 512, 512
F32 = mybir.dt.float32
BF16 = mybir.dt.bfloat16
ADD = mybir.AluOpType.add
SUB = mybir.AluOpType.subtract

_CACHED = {}


def _build(C=C, H=H, W=W, KC=4, P=128):
    HO, WO = H // 2, W // 2
    N_HB = HO // P
    nc = bacc.Bacc("TRN2", target_bir_lowering=False, debug=False)
    x = nc.dram_tensor("x", [C, H, W], F32, kind="ExternalInput").ap()
    out = nc.dram_tensor("out", [4 * C, HO, WO], F32, kind="ExternalOutput").ap()

    blocks = [(cg * KC, hb * P) for cg in range(C // KC) for hb in range(N_HB)]

    with tile.TileContext(nc) as tc, ExitStack() as ctx:
        xpool = ctx.enter_context(tc.tile_pool(name="xp", bufs=5))
        hpool = ctx.enter_context(tc.tile_pool(name="hf", bufs=2))
        mpool = ctx.enter_context(tc.tile_pool(name="mid", bufs=2))
        rpool = ctx.enter_context(tc.tile_pool(name="raw", bufs=4))

        pending = None  # (s_t, d_t, c0, i0) awaiting stage2 + halve + store

        def stage2_and_store(s_t, d_t, c0, i0):
            s2 = s_t[:].rearrange("p (k j t) -> p k j t", k=KC, t=2)
            d2 = d_t[:].rearrange("p (k j t) -> p k j t", k=KC, t=2)
            s_e, s_o = s2[:, :, :, 0], s2[:, :, :, 1]
            d_e, d_o = d2[:, :, :, 0], d2[:, :, :, 1]

            rt = rpool.tile([P, KC * 4 * WO], F32)
            r4 = rt[:].rearrange("p (k q j) -> p k q j", k=KC, q=4)
            nc.vector.tensor_tensor(r4[:, :, 0, :], s_e, s_o, ADD)  # ll
            nc.vector.tensor_tensor(r4[:, :, 1, :], d_e, d_o, ADD)  # lh
            nc.vector.tensor_tensor(r4[:, :, 2, :], s_o, s_e, SUB)  # hl
            nc.vector.tensor_tensor(r4[:, :, 3, :], d_o, d_e, SUB)  # hh

            # One store DMA: the block's 4*KC output channels are
            # contiguous in DRAM.
            dst = out[4 * c0 : 4 * (c0 + KC), i0 : i0 + P, :].transpose([1, 0, 2])
            nc.sync.dma_start(dst, rt[:].rearrange("p (c j) -> p c j", j=WO))

        for c0, i0 in blocks:
            # ---- load: (128, KC, 1024); p holds rows 2*(i0+p), 2*(i0+p)+1
            xt = xpool.tile([P, KC * 2 * W], F32)
            src = x[c0 : c0 + KC, 2 * i0 : 2 * i0 + 2 * P, :].rearrange(
                "k (p t) w -> p k (t w)", t=2
            )
            nc.scalar.dma_start(xt[:].rearrange("p (k f) -> p k f", k=KC), src)

            # ---- halve + cast f32->bf16 (DVE tensor_scalar, 2x mode).
            # bf16 mids put the vertical butterfly in 2x_1P mode and
            # halve the mid-tensor SBUF traffic; rel err ~4e-3 < 2e-2.
            xh = hpool.tile([P, KC * 2 * W], BF16)
            nc.vector.tensor_scalar_mul(xh[:], xt[:], 0.5)

            x4 = xh[:].rearrange("p (k t w) -> p k t w", k=KC, t=2)
            top, bot = x4[:, :, 0, :], x4[:, :, 1, :]

            # ---- vertical butterfly (DVE, bf16 2x)
            s_t = mpool.tile([P, KC * W], BF16)
            d_t = mpool.tile([P, KC * W], BF16)
            sv = s_t[:].rearrange("p (k w) -> p k w", k=KC)
            dv = d_t[:].rearrange("p (k w) -> p k w", k=KC)
            nc.vector.tensor_tensor(sv, top, bot, ADD)
            nc.vector.tensor_tensor(dv, bot, top, SUB)

            # ---- previous block's stage2 + halve + store (software pipelining)
            if pending is not None:
                stage2_and_store(*pending)
            pending = (s_t, d_t, c0, i0)

        stage2_and_store(*pending)
    nc.compile()
    return nc


def _get_nc():
    if "nc" not in _CACHED:
        _CACHED["nc"] = _build()
    return _CACHED["nc"]


def _run(x, **kwargs):
    x = np.ascontiguousarray(np.asarray(x), dtype=np.float32)
    assert x.shape == (N_CORES, C, H, W), x.shape
    nc = _get_nc()
    in_maps = [{"x": np.ascontiguousarray(x[i])} for i in range(N_CORES)]
    res = run_bass_kernel_spmd(nc, in_maps, core_ids=list(range(N_CORES)), **kwargs)
    out = np.stack([res.results[i]["out"] for i in range(N_CORES)], axis=0)
    return out, res


def kernel(x):
    return _run(x)[0]
